# revision 29
# baseline (speedup 1.0000x reference)
"""GAT (3-layer, DGL-style) on 8 Trainium2 NeuronCores — v4.

Sharding: nodes across the 8 cores (6250 each, padded to 6272 = 49*128),
per-core nodes permuted by descending in-degree.  A "window" is 128 nodes;
a node is pinned to one SBUF partition lane of its window.  Per layer:

  Phase A (node side): featT = W^T @ h^T per window on PE (bf16), el/er via a
  small second matmul, build 512-byte gather-table rows
  [feat 128xbf16 | el 4xf32] in local DRAM.  The quarter-major table is
  AllGathered in 4 window-quarter chunks interleaved with phase A.

  Phase B (edge side): per window, edge tiles of 128 edges = one in-edge per
  destination partition.  dma_gather fetches 512B source rows.  int16 index
  range is handled with 7 overlapping base windows into the table (stride
  6144 rows, each covering 32768 rows); each tile is typed by base and each
  edge is assigned to a compatible tile host-side with an
  earliest-deadline-first pass, which keeps the tile count within ~2% of the
  per-lane max in-degree bound (no lo/hi table split).  er[dst] is a
  per-partition constant.  exp(lrelu(s)-C) = max(exp(s-C), exp(0.2*s-C)) on
  ACT.  Messages (+ per-head exp columns, bf16) are segment-summed by a bf16
  identity-lhsT PE matmul accumulating into one PSUM bank per window.

C is a per-core bound lrelu(max el + max er) + 3 computed on device; shifting
exp by C instead of the per-segment max changes the reference's +1e-9 epsilon
term by < 1e-3 relative.
"""

import sys

sys.path.insert(0, "/opt/trn_rl_repo")

import numpy as np
import ml_dtypes

import concourse.bass as bass
import concourse.bacc as bacc
import concourse.mybir as mybir
import concourse.tile as tile
from concourse.bass_utils import run_bass_kernel_spmd

F32 = mybir.dt.float32
BF16 = mybir.dt.bfloat16
I16 = mybir.dt.int16
AF = mybir.ActivationFunctionType
OP = mybir.AluOpType
AX = mybir.AxisListType

NPBF = ml_dtypes.bfloat16

N_CORES = 8
DIM = 128
ROW_BF = 256           # table row stride in bf16 elems (512 B)
TBL_COLS = 136         # used cols: 128 feat bf16 + 4 el f32 (8 bf16 slots)
CAP = 16               # max tiles per dma_gather call
GRP = 16               # tiles per DVE/ACT group
NBASE = 7              # overlapping int16 index bases
BSTEP = 6144           # base stride in table rows
SWGRP = 1              # windows sharing merged gather calls
NEG_SLOPE = 0.2
C_MARGIN = 3.0
HEADS = (4, 4, 1)

QWIN = (12, 12, 12, 13)  # window-quarter split for chunked AllGather


# ---------------------------------------------------------------------------
# Host-side preprocessing
# ---------------------------------------------------------------------------

def preprocess(src, dst, n_nodes):
    src = np.asarray(src).astype(np.int64)
    dst = np.asarray(dst).astype(np.int64)
    npc = n_nodes // N_CORES
    NP = ((npc + 127) // 128) * 128
    W = NP // 128
    assert W == sum(QWIN)
    qsize = [q * 128 for q in QWIN]
    qstart = np.cumsum([0] + qsize[:-1])
    chunk_off = np.cumsum([0] + [N_CORES * s for s in qsize[:-1]])

    core = dst // npc
    local = dst - core * npc

    perm = []
    pos_of = np.empty(n_nodes, dtype=np.int64)
    for c in range(N_CORES):
        deg_c = np.bincount(local[core == c], minlength=npc)
        p = np.argsort(-deg_c, kind="stable")
        perm.append(p)
        inv = np.empty(npc, dtype=np.int64)
        inv[p] = np.arange(npc)
        pos_of[c * npc:(c + 1) * npc] = inv

    # quarter-major global table row for each node
    qidx = np.searchsorted(np.array(list(qstart[1:]) + [NP]), pos_of,
                           side="right")
    qs = np.array(qstart)[qidx]
    qz = np.array(qsize)[qidx]
    co = np.array(chunk_off)[qidx]
    node_core = np.arange(n_nodes) // npc
    row_of = co + node_core * qz + (pos_of - qs)

    seg_pos = pos_of[dst]
    wv = seg_pos // 128
    pv = seg_pos % 128

    # int16 base windows: edge with table row r usable from base k iff
    # BSTEP*k <= r <= BSTEP*k + 32767
    r = row_of[src]
    lo = np.maximum(0, -(-(r - 32767) // BSTEP))
    hi = np.minimum(NBASE - 1, r // BSTEP)
    lane = (core * W + wv) * 128 + pv

    # per-lane interval counts -> per-window typed-tile quotas T[w, k]
    ikey = lane * (NBASE * NBASE) + lo * NBASE + hi
    cntI = np.bincount(ikey, minlength=N_CORES * W * 128 * NBASE * NBASE)
    cntI = cntI.reshape(-1, NBASE, NBASE)
    crev = cntI[:, ::-1, :].cumsum(axis=1)[:, ::-1, :]
    cc = (crev.cumsum(axis=2).reshape(N_CORES, W, 128, NBASE, NBASE)
          .max(axis=(0, 2)))
    T = np.zeros((W, NBASE), np.int64)
    for w in range(W):
        for k in range(NBASE):
            best = 0
            for a in range(k + 1):
                best = max(best, cc[w, a, k] - T[w, a:k].sum())
            T[w, k] = best

    # earliest-deadline-first edge -> (base, slot) assignment
    E = len(src)
    assigned = np.zeros(E, bool)
    kass = np.full(E, -1, np.int64)
    tv = np.full(E, -1, np.int64)
    w_of_lane = (np.arange(N_CORES * W * 128) // 128) % W
    for k in range(NBASE):
        elig = (~assigned) & (lo <= k) & (k <= hi)
        idxs = np.flatnonzero(elig)
        order = idxs[np.lexsort((hi[idxs], lane[idxs]))]
        lane_o = lane[order]
        newg = np.r_[True, lane_o[1:] != lane_o[:-1]]
        starts = np.flatnonzero(newg)
        gidx = np.cumsum(newg) - 1
        cumc = np.arange(len(order)) - starts[gidx]
        take = cumc < T[w_of_lane[lane_o], k]
        sel = order[take]
        assigned[sel] = True
        kass[sel] = k
        tv[sel] = cumc[take]
    assert assigned.all()

    gtot = int(T.sum())
    tile_off = np.zeros((W, NBASE), dtype=np.int64)
    acc = 0
    for w in range(W):
        for k in range(NBASE):
            tile_off[w, k] = acc
            acc += T[w, k]

    # merge calls across SW-window groups sharing the same base: each call is
    # (base k, [(window, global tile slot), ...]) with <= CAP tiles
    calls = []
    for s in range(0, W, SWGRP):
        ws = range(s, min(s + SWGRP, W))
        for k in range(NBASE):
            tl = [(w, int(tile_off[w, k]) + t)
                  for w in ws for t in range(int(T[w, k]))]
            for i in range(0, len(tl), CAP):
                calls.append((k, tl[i:i + CAP]))
    icols = 8 * sum(len(tl) for (_, tl) in calls)

    idx_imgs, valids = [], []
    for c in range(N_CORES):
        m = core == c
        slots_idx = np.zeros((128, gtot), dtype=np.int64)
        slots_val = np.zeros((128, gtot), dtype=np.float32)
        g = tile_off[wv[m], kass[m]] + tv[m]
        slots_idx[pv[m], g] = r[m] - kass[m] * BSTEP
        slots_val[pv[m], g] = 1.0
        img = np.zeros((16, icols), dtype=np.int16)
        colp = 0
        for (k, tl) in calls:
            nt = len(tl)
            part = slots_idx[:, [g for (_, g) in tl]]   # [128, nt]
            flat = part.T.reshape(-1)                   # j = t*128 + p
            img[:, colp:colp + nt * 8] = flat.reshape(nt * 8, 16).T
            colp += nt * 8
        idx_imgs.append(np.ascontiguousarray(np.tile(img, (8, 1))))
        valids.append(slots_val.astype(NPBF))

    return dict(perm=perm, calls=calls, T=T,
                idx_img=idx_imgs, valid=valids, NP=NP, W=W, gtot=gtot,
                icols=icols, npc=npc, tile_off=tile_off,
                qsize=qsize, qstart=list(qstart), chunk_off=list(chunk_off))


def pack_weights(Wl, al, ar):
    H, Dh = Wl.shape[1], Wl.shape[2]
    Wm = np.ascontiguousarray(np.asarray(Wl, dtype=np.float32)
                              .reshape(Wl.shape[0], H * Dh))
    A = np.zeros((H * Dh, 8), dtype=np.float32)
    for h in range(H):
        A[h * Dh:(h + 1) * Dh, h] = np.asarray(al, dtype=np.float32)[h]
        A[h * Dh:(h + 1) * Dh, 4 + h] = np.asarray(ar, dtype=np.float32)[h]
    return Wm.astype(NPBF), A.astype(NPBF)


# ---------------------------------------------------------------------------
# Device kernel
# ---------------------------------------------------------------------------

def build_nc(meta):
    NP, W, gtot, icols = meta["NP"], meta["W"], meta["gtot"], meta["icols"]
    calls = meta["calls"]
    NTOT = N_CORES * NP
    tile_off = meta["tile_off"]
    T = meta["T"]
    qsize, qstart, chunk_off = meta["qsize"], meta["qstart"], meta["chunk_off"]
    qlastw = np.cumsum(QWIN) - 1

    nc = bacc.Bacc(None, target_bir_lowering=False, debug=False,
                   num_devices=N_CORES, num_swdge_queues=4)

    hT0 = nc.declare_dram_parameter("hT0", [128, NP], BF16, isOutput=False)
    idx_p = nc.declare_dram_parameter("idx", [128, icols], I16, isOutput=False)
    val_p = nc.declare_dram_parameter("valid", [128, gtot], BF16,
                                      isOutput=False)
    Wp = [nc.declare_dram_parameter(f"W{l}", [128, 128], BF16, isOutput=False)
          for l in range(3)]
    Ap = [nc.declare_dram_parameter(f"A{l}", [128, 8], BF16, isOutput=False)
          for l in range(3)]
    identF_p = nc.declare_dram_parameter("identF", [128, 128], F32,
                                         isOutput=False)
    identB_p = nc.declare_dram_parameter("identB", [128, 128], BF16,
                                         isOutput=False)
    ones_p = nc.declare_dram_parameter("ones1", [1, 128], F32, isOutput=False)
    onescol_p = nc.declare_dram_parameter("onescol", [128, 1], F32,
                                          isOutput=False)
    out_p = nc.declare_dram_parameter("out", [NP, 128], F32, isOutput=True)

    with tile.TileContext(nc) as tc:
        with (
            tc.tile_pool(name="const", bufs=1) as constp,
            tc.tile_pool(name="persist", bufs=1) as pers,
            tc.tile_pool(name="featg", bufs=8) as fgp,
            tc.tile_pool(name="mext", bufs=8) as mxp,
            tc.tile_pool(name="small", bufs=12) as smp,
            tc.tile_pool(name="psum", bufs=3, space="PSUM") as psp,
            tc.tile_pool(name="psacc", bufs=2, space="PSUM") as psaccp,
            tc.tile_pool(name="dram", bufs=1, space="DRAM") as dramp,
        ):
            identF = constp.tile([128, 128], F32, tag="identF")
            nc.sync.dma_start(identF[:], identF_p[:, :])
            identB = constp.tile([128, 128], BF16, tag="identB")
            nc.sync.dma_start(identB[:], identB_p[:, :])
            ones1 = constp.tile([1, 128], F32, tag="ones1")
            nc.sync.dma_start(ones1[:], ones_p[:, :])
            onescol = constp.tile([128, 1], F32, tag="onescol")
            nc.sync.dma_start(onescol[:], onescol_p[:, :])
            Wt = [constp.tile([128, 128], BF16, tag=f"W{l}", name=f"Wt{l}")
                  for l in range(3)]
            At = [constp.tile([128, 8], BF16, tag=f"A{l}", name=f"At{l}")
                  for l in range(3)]
            for l in range(3):
                nc.sync.dma_start(Wt[l][:], Wp[l][:, :])
                nc.sync.dma_start(At[l][:], Ap[l][:, :])
            idx_sb = pers.tile([128, icols], I16, tag="idx")
            nc.sync.dma_start(idx_sb[:], idx_p[:, :])
            valid_sb = pers.tile([128, gtot], BF16, tag="valid")
            nc.sync.dma_start(valid_sb[:], val_p[:, :])

            hT = [pers.tile([128, W, 128], BF16, tag=f"hT{i}", name=f"hT{i}")
                  for i in range(2)]
            nc.sync.dma_start(hT[0][:, :, :],
                              hT0[:, :].rearrange("p (w n) -> p w n", w=W))

            elerB = pers.tile([128, W, 8], F32, tag="elerB")
            rowimg = pers.tile([128, W, TBL_COLS], BF16, tag="rowimg")

            loc_tbl = [dramp.tile([NP, ROW_BF], BF16, tag=f"loctbl{i}",
                                  name=f"loctbl{i}") for i in range(2)]
            full_tbl = [dramp.tile([NTOT, ROW_BF], BF16, tag=f"fulltbl{i}",
                                   name=f"fulltbl{i}") for i in range(2)]

            def emit_A(layer, w):
                """Node-side work for one window: project, el/er, table row."""
                loc = loc_tbl[layer % 2]
                hcur = hT[layer % 2]
                featT_ps = psp.tile([128, 128], F32, tag="ps",
                                    name="featT_ps")
                nc.tensor.matmul(featT_ps[:], Wt[layer][:],
                                 hcur[:, w, :], start=True, stop=True)
                featT_sb = smp.tile([128, 128], BF16, tag="featT_sb")
                nc.vector.tensor_copy(featT_sb[:], featT_ps[:])
                elerT_ps = psp.tile([8, 128], F32, tag="ps",
                                    name="elerT_ps")
                nc.tensor.matmul(elerT_ps[:], At[layer][:], featT_sb[:],
                                 start=True, stop=True)
                elerT_sb = smp.tile([8, 128], F32, tag="elerT_sb")
                nc.vector.tensor_copy(elerT_sb[:], elerT_ps[:])
                eler_ps = psp.tile([128, 8], F32, tag="ps", name="eler_ps")
                nc.tensor.matmul(eler_ps[:], elerT_sb[:],
                                 identF[0:8, 0:8], is_transpose=True,
                                 start=True, stop=True)
                nc.vector.tensor_copy(elerB[:, w, :], eler_ps[:])
                feat_ps = psp.tile([128, 128], BF16, tag="psb",
                                   name="feat_ps")
                nc.tensor.matmul(feat_ps[:], featT_sb[:], identB[:, :],
                                 is_transpose=True, start=True, stop=True)
                nc.vector.tensor_copy(rowimg[:, w, 0:128], feat_ps[:])
                nc.vector.tensor_copy(
                    rowimg[:, w, 128:TBL_COLS].bitcast(F32),
                    eler_ps[:, 0:4])
                nc.sync.dma_start(
                    loc[:].rearrange("(w p) f -> w p f", p=128)
                    [w, :, 0:TBL_COLS],
                    rowimg[:, w, :])

            def emit_AG(layer, q):
                """AllGather one window-quarter of the layer's table."""
                loc = loc_tbl[layer % 2]
                full = full_tbl[layer % 2]
                nc.gpsimd.collective_compute(
                    "AllGather", OP.bypass,
                    replica_groups=[list(range(N_CORES))],
                    ins=[loc[qstart[q]:qstart[q] + qsize[q], :].opt()],
                    outs=[full[chunk_off[q]:
                               chunk_off[q] + N_CORES * qsize[q], :].opt()])

            def emit_A0_pair(w0, nw):
                """Layer-0 node side for a pair of windows with batched
                projection matmuls (startup is serial, so fewer PE ops)."""
                hcur = hT[0]
                n = 128 * nw
                featT_ps = psp.tile([128, 256], F32, tag="ps",
                                    name="featT_ps2")
                nc.tensor.matmul(featT_ps[:, 0:n], Wt[0][:],
                                 hcur[:, w0:w0 + nw, :]
                                 .rearrange("p w n -> p (w n)"),
                                 start=True, stop=True)
                featT_sb = smp.tile([128, 256], BF16, tag="featT_sb2")
                nc.vector.tensor_copy(featT_sb[:, 0:n], featT_ps[:, 0:n])
                elerT_ps = psp.tile([8, 256], F32, tag="ps",
                                    name="elerT_ps2")
                nc.tensor.matmul(elerT_ps[:, 0:n], At[0][:],
                                 featT_sb[:, 0:n], start=True, stop=True)
                elerT_sb = smp.tile([8, 256], F32, tag="elerT_sb2")
                nc.vector.tensor_copy(elerT_sb[:, 0:n], elerT_ps[:, 0:n])
                for i in range(nw):
                    w = w0 + i
                    eler_ps = psp.tile([128, 8], F32, tag="ps",
                                       name="eler_ps")
                    nc.tensor.matmul(eler_ps[:],
                                     elerT_sb[:, 128 * i:128 * (i + 1)],
                                     identF[0:8, 0:8], is_transpose=True,
                                     start=True, stop=True)
                    nc.vector.tensor_copy(elerB[:, w, :], eler_ps[:])
                    feat_ps = psp.tile([128, 128], BF16, tag="psb",
                                       name="feat_ps")
                    nc.tensor.matmul(feat_ps[:],
                                     featT_sb[:, 128 * i:128 * (i + 1)],
                                     identB[:, :], is_transpose=True,
                                     start=True, stop=True)
                    nc.vector.tensor_copy(rowimg[:, w, 0:128], feat_ps[:])
                    nc.vector.tensor_copy(
                        rowimg[:, w, 128:TBL_COLS].bitcast(F32),
                        eler_ps[:, 0:4])
                    nc.sync.dma_start(
                        loc_tbl[0][:].rearrange("(w p) f -> w p f", p=128)
                        [w, :, 0:TBL_COLS],
                        rowimg[:, w, :])

            # layer-0 node side runs up front
            agq = 0
            w0 = 0
            while w0 < W:
                nw = min(2, W - w0)
                emit_A0_pair(w0, nw)
                for w in range(w0, w0 + nw):
                    if agq < 4 and w == qlastw[agq]:
                        emit_AG(0, agq)
                        agq += 1
                w0 += nw

            for layer in range(3):
                H = HEADS[layer]
                D = 128 // H
                hnext = hT[(layer + 1) % 2]

                # ---- -C = -(lrelu(max el + max er) + margin) ----
                mx = smp.tile([128, 2], F32, tag="mx")
                nc.vector.tensor_reduce(mx[:, 0:1], elerB[:, :, 0:H],
                                        axis=AX.XY, op=OP.max)
                nc.vector.tensor_reduce(mx[:, 1:2], elerB[:, :, 4:4 + H],
                                        axis=AX.XY, op=OP.max)
                mxT_ps = psp.tile([2, 128], F32, tag="ps")
                nc.tensor.matmul(mxT_ps[:], mx[:], identF[:, :],
                                 is_transpose=True, start=True, stop=True)
                mm = smp.tile([2, 1], F32, tag="mm")
                nc.vector.tensor_reduce(mm[:], mxT_ps[:, :], axis=AX.X,
                                        op=OP.max)
                s_ps = psp.tile([1, 1], F32, tag="ps")
                nc.tensor.matmul(s_ps[:], mm[:], onescol[0:2, 0:1],
                                 start=True, stop=True)
                cs = smp.tile([1, 4], F32, tag="cs")
                nc.vector.tensor_copy(cs[:, 0:1], s_ps[:])
                nc.vector.tensor_scalar(cs[:, 1:2], cs[:, 0:1], NEG_SLOPE,
                                        None, op0=OP.mult)
                nc.vector.tensor_tensor(cs[:, 2:3], cs[:, 0:1],
                                        cs[:, 1:2], op=OP.max)
                nc.vector.tensor_scalar(cs[:, 3:4], cs[:, 2:3], -1.0,
                                        -C_MARGIN, op0=OP.mult,
                                        op1=OP.add)
                negC_ps = psp.tile([128, 1], F32, tag="ps")
                nc.tensor.matmul(negC_ps[:], ones1[:], cs[:, 3:4],
                                 start=True, stop=True)
                negC = smp.tile([128, 1], F32, tag="negC")
                nc.vector.tensor_copy(negC[:], negC_ps[:])

                # ======== Phase B ========
                src_aps = [full_tbl[layer % 2][BSTEP * k:
                                               min(BSTEP * k + 32768, NTOT),
                                               :]
                           for k in range(NBASE)]
                agq2 = 0
                colp = 0
                acc_t = {}
                first_w = {}
                done_w = {w: 0 for w in range(W)}
                ntiles_w = {w: int(T[w].sum()) for w in range(W)}
                qn = 0
                for (k, tl) in calls:
                    nt = len(tl)
                    fg = fgp.tile([128, CAP, ROW_BF], BF16, tag="fg")
                    nc.gpsimd.dma_gather(
                        fg[:, 0:nt, :], src_aps[k],
                        idx_sb[:, colp:colp + nt * 8],
                        nt * 128, nt * 128, ROW_BF, elem_step=ROW_BF,
                        single_packet=False, queue_num=qn)
                    qn = (qn + 1) % 4
                    colp += nt * 8

                    t = 0
                    while t < nt:
                        w, g0 = tl[t]
                        g = 1
                        while (t + g < nt and g < GRP
                               and tl[t + g][0] == w):
                            g += 1
                        if w not in acc_t:
                            acc_t[w] = psaccp.tile([128, 132], F32,
                                                   tag="acc",
                                                   name=f"acc{w}")
                            first_w[w] = True
                        acc_ps = acc_t[w]
                        sx = smp.tile([128, GRP, 4], F32, tag="sx")
                        ux = smp.tile([128, GRP, 4], BF16, tag="ux")
                        ex = smp.tile([128, GRP, 4], BF16, tag="exx")
                        er_b = (elerB[:, w, 4:4 + H].unsqueeze(1)
                                .broadcast_to([128, g, H]))
                        nc.vector.tensor_tensor(
                            sx[:, 0:g, 0:H],
                            fg[:, t:t + g, 128:128 + 2 * H].bitcast(F32),
                            er_b, op=OP.add)
                        nc.scalar.activation(ux[:, 0:g, 0:H], sx[:, 0:g, 0:H],
                                             AF.Exp, bias=negC[:, 0:1],
                                             scale=1.0)
                        nc.scalar.activation(ex[:, 0:g, 0:H], sx[:, 0:g, 0:H],
                                             AF.Exp, bias=negC[:, 0:1],
                                             scale=NEG_SLOPE)
                        val_b = (valid_sb[:, g0:g0 + g].unsqueeze(2)
                                 .broadcast_to([128, g, H]))
                        nc.vector.scalar_tensor_tensor(
                            ex[:, 0:g, 0:H], ux[:, 0:g, 0:H], 1.0,
                            ex[:, 0:g, 0:H], op0=OP.mult, op1=OP.max)
                        mext = mxp.tile([128, GRP, 132], BF16, tag="mext")
                        nc.vector.tensor_tensor(mext[:, 0:g, 128:128 + H],
                                                ex[:, 0:g, 0:H], val_b,
                                                op=OP.mult)
                        ex_b = (mext[:, 0:g, 128:128 + H].unsqueeze(3)
                                .broadcast_to([128, g, H, D]))
                        nc.vector.tensor_tensor(
                            mext[:, 0:g, 0:128]
                            .rearrange("p g (h d) -> p g h d", h=H),
                            fg[:, t:t + g, 0:128]
                            .rearrange("p g (h d) -> p g h d", h=H),
                            ex_b, op=OP.mult)
                        for kk in range(g):
                            done_w[w] += 1
                            nc.tensor.matmul(
                                acc_ps[:, 0:128 + H], identB[:, :],
                                mext[:, kk, 0:128 + H],
                                start=first_w[w],
                                stop=(done_w[w] == ntiles_w[w]))
                            first_w[w] = False
                        t += g

                        if done_w[w] != ntiles_w[w]:
                            continue
                        del acc_t[w]
                        dn = smp.tile([128, 8], F32, tag="dn")
                        nc.vector.tensor_scalar(dn[:, 0:H],
                                                acc_ps[:, 128:128 + H],
                                                1e-9, None, op0=OP.add)
                        nc.vector.reciprocal(dn[:, 4:4 + H], dn[:, 0:H])
                        hsb = smp.tile([128, 128], F32, tag="hsb")
                        rec_b = (dn[:, 4:4 + H].unsqueeze(2)
                                 .broadcast_to([128, H, D]))
                        nc.vector.tensor_tensor(
                            hsb[:].rearrange("p (h d) -> p h d", h=H),
                            acc_ps[:, 0:128]
                            .rearrange("p (h d) -> p h d", h=H),
                            rec_b, op=OP.mult)
                        if layer < 2:
                            hT_ps = psp.tile([128, 128], F32, tag="ps")
                            nc.tensor.matmul(hT_ps[:], hsb[:], identF[:, :],
                                             is_transpose=True,
                                             start=True, stop=True)
                            nc.scalar.activation(hnext[:, w, :], hT_ps[:],
                                                 AF.Relu)
                            # fuse next layer's node-side work for this
                            # window so phase A hides behind phase B
                            emit_A(layer + 1, w)
                            if w == qlastw[agq2]:
                                emit_AG(layer + 1, agq2)
                                agq2 += 1
                        else:
                            nc.sync.dma_start(
                                out_p[:, :].rearrange("(w p) f -> w p f",
                                                      p=128)[w, :, :],
                                hsb[:])
    nc.finalize()
    return nc


# ---------------------------------------------------------------------------
# Entry point
# ---------------------------------------------------------------------------

def kernel(features, src, dst, W0, al0, ar0, W1, al1, ar1, W2, al2, ar2):
    out, _ = run_gat(features, src, dst, W0, al0, ar0, W1, al1, ar1,
                     W2, al2, ar2, trace=False)
    return out


def run_gat(features, src, dst, W0, al0, ar0, W1, al1, ar1, W2, al2, ar2,
            trace=False):
    features = np.asarray(features, dtype=np.float32)
    n_nodes = features.shape[0]
    meta = preprocess(src, dst, n_nodes)
    NP, W, npc = meta["NP"], meta["W"], meta["npc"]

    Wm0, A0 = pack_weights(np.asarray(W0), al0, ar0)
    Wm1, A1 = pack_weights(np.asarray(W1), al1, ar1)
    Wm2, A2 = pack_weights(np.asarray(W2), al2, ar2)

    identF = np.eye(128, dtype=np.float32)
    identB = np.eye(128, dtype=np.float32).astype(NPBF)
    ones1 = np.ones((1, 128), dtype=np.float32)
    onescol = np.ones((128, 1), dtype=np.float32)

    in_maps = []
    for c in range(N_CORES):
        h_c = np.zeros((NP, 128), dtype=np.float32)
        h_c[:npc] = features[c * npc:(c + 1) * npc][meta["perm"][c]]
        in_maps.append({
            "hT0": np.ascontiguousarray(h_c.T).astype(NPBF),
            "idx": meta["idx_img"][c],
            "valid": meta["valid"][c],
            "W0": Wm0, "W1": Wm1, "W2": Wm2,
            "A0": A0, "A1": A1, "A2": A2,
            "identF": identF, "identB": identB,
            "ones1": ones1, "onescol": onescol,
        })

    nc = build_nc(meta)
    br = run_bass_kernel_spmd(nc, in_maps, list(range(N_CORES)), trace=trace)
    res = br.results

    out = np.empty((n_nodes, 128), dtype=np.float32)
    for c in range(N_CORES):
        o = np.asarray(res[c]["out"])
        out[c * npc:(c + 1) * npc] = o[np.argsort(meta["perm"][c])]
    return out, br
